# revision 15
# baseline (speedup 1.0000x reference)
"""Trainium2 Bass kernel for nn_CachedCompressedLinear.

out[16, 11008] = x[16, 4096] @ ((w_q - 128) * scale).T + bias

Sharding: column-parallel over 8 NeuronCores; each core computes a
[16, 1376] slice of the output (11008 = 8 * 1376).

vs the int32 baseline: the quantized codes fit in ONE byte, so the host
re-encodes w_q as (w_q - 128).astype(int8) -- lossless -- in a
per-partition-contiguous layout, cutting HBM traffic 4x (22.5 MB ->
5.63 MB per core).  On device the decode is a pure int8 -> bf16 copy
(codes |v| <= 128 are exact in bf16) split between DVE and ACT only:
GpSimd is kept OFF the decode path because its SBUF port is the shared
pair DVE's 2x/4x perf modes need (exclusive lock, loser fully blocks).
x is replicated in bf16 (error ~1.7e-3 relative, tolerance 2e-2).
Bias is added in the epilogue from a host-replicated [16, 1376] f32
tile; the scale is fused into the epilogue ACT copy.  Warmup matmuls
on a memset tile keep the PE HAM clock-gate warm until real weights
arrive.
"""

import sys

if "/opt/trn_rl_repo" not in sys.path:
    sys.path.insert(0, "/opt/trn_rl_repo")

import numpy as np
import ml_dtypes

IN_F = 4096
OUT_F = 11008
BATCH = 16
N_CORES = 8
O_PER = 1376  # out_features per core (11008 = 8 * 1376)
K_TILES = 32  # 4096 / 128
M = 16  # stationary columns: x in bf16
CHUNKS = [(0, 512), (512, 512), (1024, 352)]  # PSUM-bank-sized o-chunks

DVE_W = 1032  # decode split: DVE cols [0:1032), ACT cols [1032:1376)

# weight DMA schedule over the first 30 k-tiles: small groups first for
# fast pipeline startup, then quads; k30/k31 are DMAed chunk-wise so each
# output chunk can close as soon as its own tail slice lands.
SCHED = [(0, 1), (1, 1), (2, 2), (4, 4), (8, 4), (12, 4), (16, 4), (20, 4),
         (24, 4), (28, 2)]
K_TAIL = [30, 31]
TAIL_ENG = {0: "v", 1: "a", 2: "v"}  # chunk -> decode engine for the tail

N_WARM = 22  # PE warmup matmuls (128 cols each) to absorb the HAM ramp

_BUILT = None


def _build():
    """Build the (SPMD, per-core) Bass program once."""
    import concourse.bass as bass
    import concourse.tile as tile
    from concourse import bacc, mybir

    dt = mybir.dt
    nc = bacc.Bacc("TRN2", target_bir_lowering=False, debug=False)

    w8 = nc.dram_tensor("w8", [128, K_TILES, O_PER], dt.int8,
                        kind="ExternalInput")
    xt = nc.dram_tensor("xt", [128, K_TILES * M], dt.bfloat16,
                        kind="ExternalInput")
    bias_rep = nc.dram_tensor("bias_rep", [BATCH, O_PER], dt.float32,
                              kind="ExternalInput")
    s128 = nc.dram_tensor("s128", [128, 1], dt.float32,
                          kind="ExternalInput")
    out = nc.dram_tensor("out", [BATCH, O_PER], dt.float32,
                         kind="ExternalOutput")

    with tile.TileContext(nc) as tc:
        with (
            tc.tile_pool(name="consts", bufs=1) as consts,
            tc.tile_pool(name="w8p", bufs=1) as w8p,
            tc.tile_pool(name="wbfp", bufs=1) as wbfp,
            tc.tile_pool(name="psum", bufs=1, space=bass.MemorySpace.PSUM) as psump,
            tc.tile_pool(name="outp", bufs=1) as outp,
        ):
            def _copy(e, dst, src):
                # decode fuses the dequant scale: wbf = bf16(s * code).
                # ACT has no tensor_copy; a scaled Copy activation matches.
                if e == "a":
                    nc.scalar.activation(
                        dst, src, mybir.ActivationFunctionType.Copy,
                        scale=s_sb[:, 0:1])
                else:
                    nc.vector.tensor_scalar_mul(dst, src, s_sb[:, 0:1])

            # ---- x on SP first (fast HWDGE path; gates all matmuls);
            # bias/scale ride the Pool/SWDGE path which is otherwise idle
            x_sb = consts.tile([128, K_TILES * M], dt.bfloat16)
            nc.sync.dma_start(x_sb[:], xt[:])
            bias_sb = consts.tile([BATCH, O_PER], dt.float32)
            nc.gpsimd.dma_start(bias_sb[:], bias_rep[:])
            s_sb = consts.tile([128, 1], dt.float32)
            nc.gpsimd.dma_start(s_sb[:], s128[:])

            # ---- weight stream on SP/HWDGE
            w8_t = {}
            for k0, nk in SCHED:
                t = w8p.tile([128, nk, O_PER], dt.int8, tag=f"w8_{k0}")
                nc.sync.dma_start(t[:], w8[:][:, k0:k0 + nk, :])
                w8_t[k0] = t
            # tail: k30/k31 chunk-wise
            w8_tail = {}
            for i, (o, w) in enumerate(CHUNKS):
                for k in K_TAIL:
                    t = w8p.tile([128, w], dt.int8, tag=f"w8t_{i}_{k}")
                    nc.sync.dma_start(t[:], w8[:][:, k, o:o + w])
                    w8_tail[(i, k)] = t

            # ---- PE warmup: matmuls on a zeroed tile; tiny memset so the
            # dependency resolves as early as possible
            warm_mv = consts.tile([128, 128], dt.bfloat16)
            nc.vector.memset(warm_mv[:], 0.0)
            warm_ps = psump.tile([16, 128], dt.float32, tag="warm")
            for _ in range(N_WARM):
                nc.tensor.matmul(warm_ps[:], warm_mv[:, 0:16], warm_mv[:],
                                 start=True, stop=True)

            # ---- decode int8 -> bf16 (pure dtype-converting copy)
            wbf_t = {}
            for k0, nk in SCHED:
                t = wbfp.tile([128, nk, O_PER], dt.bfloat16, tag=f"wbf_{k0}")
                wbf_t[k0] = t
            for k0, nk in SCHED:
                # DVE per single k (prompt release of matmuls)
                for j in range(nk):
                    _copy("v", wbf_t[k0][:, j, 0:DVE_W],
                          w8_t[k0][:, j, 0:DVE_W])
                # ACT per k-pair (its per-instruction init overhead is high)
                step = 2 if nk >= 2 else 1
                for j in range(0, nk, step):
                    js = slice(j, j + step)
                    _copy("a", wbf_t[k0][:, js, DVE_W:O_PER],
                          w8_t[k0][:, js, DVE_W:O_PER])
            wbf_tail = {}
            for i, (o, w) in enumerate(CHUNKS):
                for k in K_TAIL:
                    t = wbfp.tile([128, w], dt.bfloat16, tag=f"wbft_{i}_{k}")
                    _copy(TAIL_ENG[i], t[:], w8_tail[(i, k)][:])
                    wbf_tail[(i, k)] = t

            # ---- matmuls
            psums = [
                psump.tile([16, w], dt.float32, name=f"ps{i}", tag=f"ps{i}")
                for i, (_, w) in enumerate(CHUNKS)
            ]
            for k0, nk in SCHED:
                for j in range(nk):
                    k = k0 + j
                    for i, (o, w) in enumerate(CHUNKS):
                        nc.tensor.matmul(
                            psums[i][:],
                            x_sb[:, k * M:(k + 1) * M],
                            wbf_t[k0][:, j, o:o + w],
                            start=(k == 0),
                            stop=False,
                        )
            # tail, chunk-major so each chunk closes in turn
            for i, (o, w) in enumerate(CHUNKS):
                for k in K_TAIL:
                    nc.tensor.matmul(
                        psums[i][:],
                        x_sb[:, k * M:(k + 1) * M],
                        wbf_tail[(i, k)][:],
                        start=False,
                        stop=(k == K_TAIL[-1]),
                    )

            # ---- epilogue per chunk: PSUM already holds s*(x@W), so one
            # DVE add of the f32 bias closes the chunk; SP DMAs it out.
            for i, (o, w) in enumerate(CHUNKS):
                comb = outp.tile([BATCH, w], dt.float32, tag=f"comb_{i}")
                nc.vector.tensor_add(comb[:], psums[i][:], bias_sb[:, o:o + w])
                nc.sync.dma_start(out[:][:, o:o + w], comb[:])

    nc.compile()
    return nc


def _get_built():
    global _BUILT
    if _BUILT is None:
        _BUILT = _build()
    return _BUILT


def make_in_maps(x, w_q, scale, bias):
    """Host-side shard + layout prep. Returns per-core input dicts."""
    x = np.asarray(x, dtype=np.float32)
    w_q = np.asarray(w_q, dtype=np.int32)
    scale = np.asarray(scale, dtype=np.float32)
    bias = np.asarray(bias, dtype=np.float32)

    # x -> bf16, packed so partition p holds, for each k-tile t, the
    # stationary row (t*128 + p): [128, 32*16]
    xT = np.ascontiguousarray(x.T).astype(ml_dtypes.bfloat16)  # [4096, 16]
    xt = np.ascontiguousarray(
        xT.reshape(K_TILES, 128, M).transpose(1, 0, 2)
    ).reshape(128, K_TILES * M)

    # codes -> int8 (lossless: w_q in [0,255], shift to [-128,127])
    w8_full = (w_q - 128).astype(np.int8)  # [11008, 4096]

    s_val = scale.reshape(-1)[0]
    s128 = np.full((128, 1), s_val, dtype=np.float32)

    in_maps = []
    for c in range(N_CORES):
        sl = w8_full[c * O_PER:(c + 1) * O_PER]  # [1376, 4096]
        # [128, 32, 1376]: partition p, (k, f) = W[f, k*128 + p]
        w8c = np.ascontiguousarray(
            sl.T.reshape(K_TILES, 128, O_PER).transpose(1, 0, 2)
        )
        bias_c = np.ascontiguousarray(
            np.broadcast_to(bias[c * O_PER:(c + 1) * O_PER], (BATCH, O_PER))
        )
        in_maps.append(
            {"w8": w8c, "xt": xt, "bias_rep": bias_c, "s128": s128}
        )
    return in_maps


def run(inputs, trace=False):
    """Run on the 8 NeuronCores. Returns (full_output, BassKernelResults)."""
    from concourse.bass_utils import run_bass_kernel_spmd

    in_maps = make_in_maps(**inputs)
    nc = _get_built()
    res = run_bass_kernel_spmd(nc, in_maps, list(range(N_CORES)), trace=trace)
    parts = [np.asarray(res.results[c]["out"]) for c in range(N_CORES)]
    full = np.concatenate(parts, axis=1)[:, :OUT_F].astype(np.float32)
    return full, res


def kernel(**inputs) -> np.ndarray:
    full, _ = run(inputs, trace=False)
    return full
